# revision 1
# baseline (speedup 1.0000x reference)
"""DSQG attention kernel for 8 Trainium2 NeuronCores.

Sharding: core c = (b, half) with b = c//2 in 0..3, half = c%2.
Each core handles batch b and heads [half*8, half*8+8):
  - qkv+gate projection for its 512 channels (bias folded in via ones-column)
  - 44-tap dyadic attention (24 distinct lags; duplicate-offset pos_bias terms
    folded into per-lag multiplicative weights W[L,h] = sum_dup exp(pos_bias))
  - gated partial output projection through its 512 rows of w_out
Host sums the two half partials per batch and adds b_out.

Shifted k/v windows are fetched per (block, lag) from a zero-padded DRAM
buffer: DRAM rows have no partition-alignment constraint, unlike SBUF reads
(start partition must be 0/32/64/96).
"""
import sys

sys.path.insert(0, "/opt/trn_rl_repo")

import numpy as np
import ml_dtypes

N_SCALES = 11
N_TAPS = 4
OFFSETS = [(1 << j) * tau for j in range(N_SCALES) for tau in range(N_TAPS)]
LAGS = sorted(set(OFFSETS))  # 24 distinct lags
NL = len(LAGS)
B, N, D, H = 4, 4096, 1024, 16
HD = 64
HH = 8  # heads per core
CW = HH * HD  # 512 channels per core
KE = 1152  # padded contraction: 1024 x-cols + 1 ones-col + 127 zero pad
NBLK = N // 128  # 32
PAD = 3072  # zero rows above qkvg for causal lookback
SCALE = HD ** -0.5

# lags needing a DMA window fetch (walrus requires identical start partitions
# on all TensorTensor operands, so only 128-multiples can read SBUF directly)
SMALL = [1, 2, 3, 4, 6, 8, 12, 16, 24, 32, 48, 64, 96, 192]

_CACHE = {}


def _pieces(L):
    """(out_lo, out_hi, tile_delta, src_lo) for lags that are 128-multiples."""
    qd, r = divmod(L, 128)
    assert r == 0, L
    return [(0, 128, qd, 0)]


def _build_program():
    import concourse.bacc as bacc
    import concourse.mybir as mybir
    import concourse.tile as tile
    from concourse.kernels.tile_matmul import matmul_tile_kernel

    bf16 = mybir.dt.bfloat16
    f32 = mybir.dt.float32

    nc = bacc.Bacc("TRN2", target_bir_lowering=False, debug=False, num_devices=8)
    xb = nc.dram_tensor("xb", [N, KE], bf16, kind="ExternalInput").ap()
    wqg = nc.dram_tensor("wqg", [KE, 4 * CW], bf16, kind="ExternalInput").ap()
    wo = nc.dram_tensor("wo", [CW, D], bf16, kind="ExternalInput").ap()
    wrep = nc.dram_tensor("wrep", [128, HH * NL], f32, kind="ExternalInput").ap()
    y = nc.dram_tensor("y", [N, D], f32, kind="ExternalOutput").ap()

    with tile.TileContext(nc) as tc:
        with (
            tc.tile_pool(name="dram", bufs=1, space="DRAM") as dram,
            tc.tile_pool(name="kvpool", bufs=1) as kvpool,
            tc.tile_pool(name="work", bufs=2) as work,
            tc.tile_pool(name="winp", bufs=16) as winp,
            tc.tile_pool(name="prodp", bufs=1) as prodp,
            tc.tile_pool(name="constp", bufs=1) as constp,
        ):
            qkvg = dram.tile([PAD + N, 4 * CW], bf16)
            gtil = dram.tile([N, CW], bf16)

            # zero the pad rows (read back by the small-lag window DMAs)
            ztile = constp.tile([128, 2 * CW], bf16, tag="ztile")
            nc.vector.memset(ztile[:], 0.0)
            for t in range(PAD // 128):
                nc.sync.dma_start(out=qkvg[t * 128:(t + 1) * 128, CW:3 * CW], in_=ztile[:])

            # Phase 1: [q|k|v|gate] = x_ext @ w_ext   (biases via ones column)
            matmul_tile_kernel(tc, xb, wqg, qkvg[PAD:, :], transpose_kxm=True)
            tc.strict_bb_all_engine_barrier()

            wr = constp.tile([128, HH * NL], f32, tag="wr")
            nc.sync.dma_start(out=wr[:], in_=wrep[:])

            # k, v resident in SBUF for the quadrant-alignable lags
            kb = kvpool.tile([128, NBLK * CW], bf16, tag="kb")
            vb = kvpool.tile([128, NBLK * CW], bf16, tag="vb")
            for a in range(NBLK):
                rws = slice(PAD + a * 128, PAD + (a + 1) * 128)
                nc.sync.dma_start(out=kb[:, a * CW:(a + 1) * CW], in_=qkvg[rws, CW:2 * CW])
                nc.sync.dma_start(out=vb[:, a * CW:(a + 1) * CW], in_=qkvg[rws, 2 * CW:3 * CW])

            GRP = 1  # blocks per small-lag window fetch group

            def fetch_small_windows(g):
                """One DMA per small lag: k and v windows for GRP blocks.

                Returns {L: tile [128, GRP*2*CW] viewed (p, a_local, kv, c)}."""
                wins = {}
                base = PAD + g * GRP * 128
                for i, L in enumerate(SMALL):
                    win = winp.tile([128, GRP * 2 * CW], bf16, tag="win")
                    eng = nc.sync if (i % 2 == 0) else nc.scalar
                    src = qkvg[base - L:base - L + GRP * 128, CW:3 * CW]
                    eng.dma_start(out=win[:], in_=src)
                    wins[L] = win
                return wins

            def tap_muls(a0, wins, prod, lhs_of, resident, kv):
                """Emit prod[:, j*CW:(j+1)*CW] = lhs ⊙ window_L for every lag.

                lhs_of(j): 3D [128, HH, HD] left operand for lag slot j.
                resident: SBUF tensor (kb/vb) for 128-multiple lags;
                kv: 0 for k windows, 1 for v windows (small lags)."""
                def r3(ap):
                    return ap.rearrange("p (h d) -> p h d", h=HH)

                al = a0 % GRP
                for j, L in enumerate(LAGS):
                    dst = prod[:, j * CW:(j + 1) * CW]
                    if L in SMALL:
                        w0 = (al * 2 + kv) * CW
                        nc.vector.tensor_mul(
                            r3(dst), lhs_of(j), r3(wins[L][:, w0:w0 + CW]))
                        continue
                    a = a0 - L // 128
                    if a < 0:
                        nc.vector.memset(dst[:, :], 0.0)
                    else:
                        nc.vector.tensor_mul(
                            r3(dst), lhs_of(j),
                            r3(resident[:, a * CW:(a + 1) * CW]))

            # Phase 2: attention middle, per 128-row block
            wins = None
            for a0 in range(NBLK):
                if a0 % GRP == 0:
                    wins = fetch_small_windows(a0 // GRP)
                n0 = a0 * 128
                rows = slice(PAD + n0, PAD + n0 + 128)
                qb = work.tile([128, CW], bf16, tag="qb")
                nc.sync.dma_start(out=qb[:], in_=qkvg[rows, 0:CW])
                prod = prodp.tile([128, NL * CW], bf16, tag="prod")

                # scores products: prod[:, j*CW:(j+1)*CW] = q * k[n-L]
                tap_muls(a0, wins, prod,
                         lambda j: qb[:].rearrange("p (h d) -> p h d", h=HH),
                         kb, 0)
                # reduce over d -> scores [128, (h, j)]
                scores = work.tile([128, HH * NL], f32, tag="scores")
                nc.vector.tensor_reduce(
                    out=scores[:].rearrange("p (h l) -> p l h", h=HH, l=NL).unsqueeze(3),
                    in_=prod[:].rearrange("p (l h d) -> p l h d", l=NL, h=HH),
                    axis=mybir.AxisListType.X,
                    op=mybir.AluOpType.add,
                )
                # softmax over lags with folded pos-bias weights, unnormalized
                ew = work.tile([128, HH * NL], f32, tag="ew")
                nc.scalar.activation(ew[:], scores[:], mybir.ActivationFunctionType.Exp, scale=SCALE)
                ew2 = work.tile([128, HH * NL], f32, tag="ew2")
                nc.vector.tensor_mul(ew2[:], ew[:], wr[:])
                z = work.tile([128, HH], f32, tag="z")
                nc.vector.tensor_reduce(
                    out=z[:].unsqueeze(2),
                    in_=ew2[:].rearrange("p (h l) -> p h l", h=HH),
                    axis=mybir.AxisListType.X,
                    op=mybir.AluOpType.add,
                )
                rz = work.tile([128, HH], f32, tag="rz")
                nc.vector.reciprocal(rz[:], z[:])
                attn = work.tile([128, HH * NL], bf16, tag="attn")
                nc.vector.tensor_mul(
                    attn[:].rearrange("p (h l) -> p h l", h=HH),
                    ew2[:].rearrange("p (h l) -> p h l", h=HH),
                    rz[:].unsqueeze(2).broadcast_to([128, HH, NL]),
                )
                # weighted taps: prod[:, j] = attn[:, :, j] * v[n-L]
                attn3 = attn[:].rearrange("p (h l) -> p h l", h=HH)

                def attn_lhs(j):
                    return attn3[:, :, j:j + 1].broadcast_to([128, HH, HD])

                tap_muls(a0, wins, prod, attn_lhs, vb, 1)
                accv = work.tile([128, CW], f32, tag="accv")
                nc.vector.tensor_reduce(
                    out=accv[:].rearrange("p (h d) -> p h d", h=HH).unsqueeze(3),
                    in_=prod[:].rearrange("p (l h d) -> p h d l", l=NL, h=HH),
                    axis=mybir.AxisListType.X,
                    op=mybir.AluOpType.add,
                )
                # gate and gathered*gate
                gt = work.tile([128, CW], bf16, tag="gt")
                nc.sync.dma_start(out=gt[:], in_=qkvg[rows, 3 * CW:4 * CW])
                gsig = work.tile([128, CW], f32, tag="gsig")
                nc.scalar.activation(gsig[:], gt[:], mybir.ActivationFunctionType.Sigmoid)
                gtl = work.tile([128, CW], bf16, tag="gtl")
                nc.vector.tensor_mul(gtl[:], accv[:], gsig[:])
                nc.sync.dma_start(out=gtil[n0:n0 + 128, :], in_=gtl[:])

            # Phase 3: y_partial = (gathered*gate) @ w_out[half rows]
            tc.strict_bb_all_engine_barrier()
            matmul_tile_kernel(tc, gtil[:], wo, y, transpose_kxm=True)

    nc.compile()
    return nc


def _get_program():
    if "nc" not in _CACHE:
        _CACHE["nc"] = _build_program()
    return _CACHE["nc"]


def _core_inputs(x, w_qkv, b_qkv, w_gate, b_gate, w_out, pos_bias, b, half):
    bf = ml_dtypes.bfloat16
    cs = slice(half * CW, (half + 1) * CW)

    xb = np.zeros((N, KE), dtype=bf)
    xb[:, :D] = x[b].astype(bf)
    xb[:, D] = np.asarray(1.0, dtype=bf)

    wqg = np.zeros((KE, 4 * CW), dtype=np.float32)
    wqg[:D, 0:CW] = w_qkv[:, cs]
    wqg[:D, CW:2 * CW] = w_qkv[:, D + cs.start:D + cs.stop]
    wqg[:D, 2 * CW:3 * CW] = w_qkv[:, 2 * D + cs.start:2 * D + cs.stop]
    wqg[:D, 3 * CW:4 * CW] = w_gate[:, cs]
    wqg[D, 0:CW] = b_qkv[cs]
    wqg[D, CW:2 * CW] = b_qkv[D + cs.start:D + cs.stop]
    wqg[D, 2 * CW:3 * CW] = b_qkv[2 * D + cs.start:2 * D + cs.stop]
    wqg[D, 3 * CW:4 * CW] = b_gate[cs]

    # lag weights: W[h, j] = sum over duplicate offsets of exp(pos_bias[i, h])
    wl = np.zeros((HH, NL), dtype=np.float32)
    for i, off in enumerate(OFFSETS):
        j = LAGS.index(off)
        wl[:, j] += np.exp(pos_bias[i, half * HH:(half + 1) * HH].astype(np.float64)).astype(np.float32)
    wrep = np.broadcast_to(wl.reshape(1, HH * NL), (128, HH * NL)).copy()

    return {
        "xb": xb,
        "wqg": wqg.astype(bf),
        "wo": w_out[cs, :].astype(bf),
        "wrep": wrep,
    }


def kernel(x, w_qkv, b_qkv, w_out, b_out, w_gate, b_gate, pos_bias):
    from concourse.bass_utils import run_bass_kernel_spmd

    x = np.asarray(x, dtype=np.float32)
    w_qkv = np.asarray(w_qkv, dtype=np.float32)
    b_qkv = np.asarray(b_qkv, dtype=np.float32)
    w_out = np.asarray(w_out, dtype=np.float32)
    b_out = np.asarray(b_out, dtype=np.float32)
    w_gate = np.asarray(w_gate, dtype=np.float32)
    b_gate = np.asarray(b_gate, dtype=np.float32)
    pos_bias = np.asarray(pos_bias, dtype=np.float32)

    nc = _get_program()
    in_maps = [
        _core_inputs(x, w_qkv, b_qkv, w_gate, b_gate, w_out, pos_bias, c // 2, c % 2)
        for c in range(8)
    ]
    res = run_bass_kernel_spmd(nc, in_maps, core_ids=list(range(8)))
    out = np.empty((B, N, D), dtype=np.float32)
    for b in range(B):
        out[b] = res.results[2 * b]["y"] + res.results[2 * b + 1]["y"] + b_out[None, :]
    return out



# revision 4
# speedup vs baseline: 1.2945x; 1.2945x over previous
"""DSQG attention kernel for 8 Trainium2 NeuronCores.

Sharding: core c = (b, half) with b = c//2 in 0..3, half = c%2.
Each core handles batch b and heads [half*8, half*8+8):
  - qkv+gate projection for its 512 channels (bias folded in via ones-column)
  - 44-tap dyadic attention (24 distinct lags; duplicate-offset pos_bias terms
    folded into per-lag multiplicative weights W[L,h] = sum_dup exp(pos_bias))
  - gated partial output projection through its 512 rows of w_out
Host sums the two half partials per batch and adds b_out.

Shifted k/v windows are fetched per (block, lag) from a zero-padded DRAM
buffer: DRAM rows have no partition-alignment constraint, unlike SBUF reads
(start partition must be 0/32/64/96).
"""
import sys

sys.path.insert(0, "/opt/trn_rl_repo")

import numpy as np
import ml_dtypes

N_SCALES = 11
N_TAPS = 4
OFFSETS = [(1 << j) * tau for j in range(N_SCALES) for tau in range(N_TAPS)]
LAGS = sorted(set(OFFSETS))  # 24 distinct lags
NL = len(LAGS)
B, N, D, H = 4, 4096, 1024, 16
HD = 64
HH = 8  # heads per core
CW = HH * HD  # 512 channels per core
KE = 1152  # padded contraction: 1024 x-cols + 1 ones-col + 127 zero pad
NBLK = N // 128  # 32
PAD = 3072  # zero rows above qkvg for causal lookback
SCALE = HD ** -0.5

# lags needing a DMA window fetch (walrus requires identical start partitions
# on all TensorTensor operands, so only 128-multiples can read SBUF directly)
SMALL = [1, 2, 3, 4, 6, 8, 12, 16, 24, 32, 48, 64, 96, 192]

_CACHE = {}


def _pieces(L):
    """(out_lo, out_hi, tile_delta, src_lo) for lags that are 128-multiples."""
    qd, r = divmod(L, 128)
    assert r == 0, L
    return [(0, 128, qd, 0)]


def _build_program():
    import concourse.bacc as bacc
    import concourse.mybir as mybir
    import concourse.tile as tile
    from concourse.kernels.tile_matmul import matmul_tile_kernel

    bf16 = mybir.dt.bfloat16
    f32 = mybir.dt.float32

    nc = bacc.Bacc("TRN2", target_bir_lowering=False, debug=False, num_devices=8)
    xb = nc.dram_tensor("xb", [N, KE], bf16, kind="ExternalInput").ap()
    wqg = nc.dram_tensor("wqg", [KE, 4 * CW], bf16, kind="ExternalInput").ap()
    wo = nc.dram_tensor("wo", [CW, D], bf16, kind="ExternalInput").ap()
    wrep = nc.dram_tensor("wrep", [128, HH * NL], f32, kind="ExternalInput").ap()
    y = nc.dram_tensor("y", [N, D], f32, kind="ExternalOutput").ap()

    with tile.TileContext(nc) as tc:
        with (
            tc.tile_pool(name="dram", bufs=1, space="DRAM") as dram,
            tc.tile_pool(name="kvpool", bufs=1) as kvpool,
            tc.tile_pool(name="work", bufs=2) as work,
            tc.tile_pool(name="winp", bufs=16) as winp,
            tc.tile_pool(name="prodp", bufs=1) as prodp,
            tc.tile_pool(name="treep", bufs=1) as treep,
            tc.tile_pool(name="constp", bufs=1) as constp,
        ):
            qkvg = dram.tile([PAD + N, 4 * CW], bf16)
            gtil = dram.tile([N, CW], bf16)

            # zero the pad rows (read back by the small-lag window DMAs)
            ztile = constp.tile([128, 2 * CW], bf16, tag="ztile")
            nc.vector.memset(ztile[:], 0.0)
            for t in range(PAD // 128):
                nc.sync.dma_start(out=qkvg[t * 128:(t + 1) * 128, CW:3 * CW], in_=ztile[:])

            # Phase 1: [q|k|v|gate] = x_ext @ w_ext   (biases via ones column)
            matmul_tile_kernel(tc, xb, wqg, qkvg[PAD:, :], transpose_kxm=True)
            tc.strict_bb_all_engine_barrier()

            wr = constp.tile([128, HH * NL], f32, tag="wr")
            nc.sync.dma_start(out=wr[:], in_=wrep[:])

            # k, v resident in SBUF for the quadrant-alignable lags
            kb = kvpool.tile([128, NBLK * CW], bf16, tag="kb")
            vb = kvpool.tile([128, NBLK * CW], bf16, tag="vb")
            for a in range(NBLK):
                rws = slice(PAD + a * 128, PAD + (a + 1) * 128)
                nc.sync.dma_start(out=kb[:, a * CW:(a + 1) * CW], in_=qkvg[rws, CW:2 * CW])
                nc.sync.dma_start(out=vb[:, a * CW:(a + 1) * CW], in_=qkvg[rws, 2 * CW:3 * CW])

            GRP = 1  # blocks per small-lag window fetch group

            def fetch_small_windows(g):
                """One DMA per small lag: k and v windows for GRP blocks.

                Returns {L: tile [128, GRP*2*CW] viewed (p, a_local, kv, c)}."""
                wins = {}
                base = PAD + g * GRP * 128
                for i, L in enumerate(SMALL):
                    win = winp.tile([128, GRP * 2 * CW], bf16, tag="win")
                    eng = nc.sync if (i % 2 == 0) else nc.scalar
                    src = qkvg[base - L:base - L + GRP * 128, CW:3 * CW]
                    eng.dma_start(out=win[:], in_=src)
                    wins[L] = win
                return wins

            def tap_muls(a0, wins, prod, lhs_of, resident, kv):
                """Emit prod[:, j*CW:(j+1)*CW] = lhs ⊙ window_L for every lag.

                lhs_of(j): 3D [128, HH, HD] left operand for lag slot j.
                resident: SBUF tensor (kb/vb) for 128-multiple lags;
                kv: 0 for k windows, 1 for v windows (small lags)."""
                def r3(ap):
                    return ap.rearrange("p (h d) -> p h d", h=HH)

                al = a0 % GRP
                for j, L in enumerate(LAGS):
                    dst = prod[:, j * CW:(j + 1) * CW]
                    if L in SMALL:
                        w0 = (al * 2 + kv) * CW
                        nc.vector.tensor_mul(
                            r3(dst), lhs_of(j), r3(wins[L][:, w0:w0 + CW]))
                        continue
                    a = a0 - L // 128
                    if a < 0:
                        nc.vector.memset(dst[:, :], 0.0)
                    else:
                        nc.vector.tensor_mul(
                            r3(dst), lhs_of(j),
                            r3(resident[:, a * CW:(a + 1) * CW]))

            # Phase 2: attention middle, per 128-row block
            wins = None
            for a0 in range(NBLK):
                if a0 % GRP == 0:
                    wins = fetch_small_windows(a0 // GRP)
                n0 = a0 * 128
                rows = slice(PAD + n0, PAD + n0 + 128)
                qb = work.tile([128, CW], bf16, tag="qb")
                nc.sync.dma_start(out=qb[:], in_=qkvg[rows, 0:CW])
                prod = prodp.tile([128, NL * CW], bf16, tag="prod")

                # scores products: prod[:, j*CW:(j+1)*CW] = q * k[n-L]
                tap_muls(a0, wins, prod,
                         lambda j: qb[:].rearrange("p (h d) -> p h d", h=HH),
                         kb, 0)
                # reduce over d -> scores [128, (l, h)] via bf16 pairwise tree
                # (tensor_tensor bf16 step-1 runs 2x mode; tensor_reduce is 1x)
                sA = treep.tile([128, 96 * HH * HD // 4], bf16, tag="sA")

                def halve(out_ap, in_ap, G, c):
                    iv = in_ap.rearrange("p (g t c) -> p g t c", g=G, t=2, c=c)
                    nc.vector.tensor_add(
                        out_ap.rearrange("p (g c) -> p g c", g=G, c=c),
                        iv[:, :, 0, :], iv[:, :, 1, :])

                GD = NL * HH  # 192 (l,h) groups
                halve(sA[:, :6144], prod[:], GD, 32)
                halve(prod[:, :3072], sA[:, :6144], GD, 16)
                halve(sA[:, :1536], prod[:, :3072], GD, 8)
                halve(prod[:, :768], sA[:, :1536], GD, 4)
                halve(sA[:, :384], prod[:, :768], GD, 2)
                scores = work.tile([128, NL * HH], f32, tag="scores")
                halve(scores[:], sA[:, :384], GD, 1)
                # softmax over lags with folded pos-bias weights, unnormalized
                ew = work.tile([128, NL * HH], f32, tag="ew")
                nc.scalar.activation(ew[:], scores[:], mybir.ActivationFunctionType.Exp, scale=SCALE)
                ew2 = work.tile([128, NL * HH], f32, tag="ew2")
                nc.vector.tensor_mul(ew2[:], ew[:], wr[:])
                z = work.tile([128, HH], f32, tag="z")
                nc.vector.tensor_reduce(
                    out=z[:].unsqueeze(2),
                    in_=ew2[:].rearrange("p (l h) -> p h l", l=NL, h=HH),
                    axis=mybir.AxisListType.X,
                    op=mybir.AluOpType.add,
                )
                rz = work.tile([128, HH], f32, tag="rz")
                nc.vector.reciprocal(rz[:], z[:])
                attn = work.tile([128, NL * HH], bf16, tag="attn")
                nc.vector.tensor_mul(
                    attn[:].rearrange("p (l h) -> p h l", l=NL, h=HH),
                    ew2[:].rearrange("p (l h) -> p h l", l=NL, h=HH),
                    rz[:].unsqueeze(2).broadcast_to([128, HH, NL]),
                )

                # weighted taps: prod[:, j] = attn[:, j, :] * v[n-L]
                def attn_lhs(j):
                    return attn[:, j * HH:(j + 1) * HH].unsqueeze(2).broadcast_to([128, HH, HD])

                tap_muls(a0, wins, prod, attn_lhs, vb, 1)
                # reduce over lags: bf16 pairwise tree over the 24 CW-chunks
                accv = work.tile([128, CW], f32, tag="accv")
                halve(sA[:, :6144], prod[:], 12, CW)
                halve(prod[:, :3072], sA[:, :6144], 6, CW)
                halve(sA[:, :1536], prod[:, :3072], 3, CW)
                nc.vector.tensor_add(prod[:, 0:CW], sA[:, 0:CW], sA[:, CW:2 * CW])
                nc.vector.tensor_add(accv[:], prod[:, 0:CW], sA[:, 2 * CW:3 * CW])
                # gate and gathered*gate
                gt = work.tile([128, CW], bf16, tag="gt")
                nc.sync.dma_start(out=gt[:], in_=qkvg[rows, 3 * CW:4 * CW])
                gsig = work.tile([128, CW], f32, tag="gsig")
                nc.scalar.activation(gsig[:], gt[:], mybir.ActivationFunctionType.Sigmoid)
                gtl = work.tile([128, CW], bf16, tag="gtl")
                nc.vector.tensor_mul(gtl[:], accv[:], gsig[:])
                nc.sync.dma_start(out=gtil[n0:n0 + 128, :], in_=gtl[:])

            # Phase 3: y_partial = (gathered*gate) @ w_out[half rows]
            tc.strict_bb_all_engine_barrier()
            matmul_tile_kernel(tc, gtil[:], wo, y, transpose_kxm=True)

    nc.compile()
    return nc


def _get_program():
    if "nc" not in _CACHE:
        _CACHE["nc"] = _build_program()
    return _CACHE["nc"]


def _core_inputs(x, w_qkv, b_qkv, w_gate, b_gate, w_out, pos_bias, b, half):
    bf = ml_dtypes.bfloat16
    cs = slice(half * CW, (half + 1) * CW)

    xb = np.zeros((N, KE), dtype=bf)
    xb[:, :D] = x[b].astype(bf)
    xb[:, D] = np.asarray(1.0, dtype=bf)

    wqg = np.zeros((KE, 4 * CW), dtype=np.float32)
    wqg[:D, 0:CW] = w_qkv[:, cs]
    wqg[:D, CW:2 * CW] = w_qkv[:, D + cs.start:D + cs.stop]
    wqg[:D, 2 * CW:3 * CW] = w_qkv[:, 2 * D + cs.start:2 * D + cs.stop]
    wqg[:D, 3 * CW:4 * CW] = w_gate[:, cs]
    wqg[D, 0:CW] = b_qkv[cs]
    wqg[D, CW:2 * CW] = b_qkv[D + cs.start:D + cs.stop]
    wqg[D, 2 * CW:3 * CW] = b_qkv[2 * D + cs.start:2 * D + cs.stop]
    wqg[D, 3 * CW:4 * CW] = b_gate[cs]

    # lag weights: W[j, h] = sum over duplicate offsets of exp(pos_bias[i, h])
    wl = np.zeros((NL, HH), dtype=np.float32)
    for i, off in enumerate(OFFSETS):
        j = LAGS.index(off)
        wl[j, :] += np.exp(pos_bias[i, half * HH:(half + 1) * HH].astype(np.float64)).astype(np.float32)
    wrep = np.broadcast_to(wl.reshape(1, NL * HH), (128, NL * HH)).copy()

    return {
        "xb": xb,
        "wqg": wqg.astype(bf),
        "wo": w_out[cs, :].astype(bf),
        "wrep": wrep,
    }


def kernel(x, w_qkv, b_qkv, w_out, b_out, w_gate, b_gate, pos_bias):
    from concourse.bass_utils import run_bass_kernel_spmd

    x = np.asarray(x, dtype=np.float32)
    w_qkv = np.asarray(w_qkv, dtype=np.float32)
    b_qkv = np.asarray(b_qkv, dtype=np.float32)
    w_out = np.asarray(w_out, dtype=np.float32)
    b_out = np.asarray(b_out, dtype=np.float32)
    w_gate = np.asarray(w_gate, dtype=np.float32)
    b_gate = np.asarray(b_gate, dtype=np.float32)
    pos_bias = np.asarray(pos_bias, dtype=np.float32)

    nc = _get_program()
    in_maps = [
        _core_inputs(x, w_qkv, b_qkv, w_gate, b_gate, w_out, pos_bias, c // 2, c % 2)
        for c in range(8)
    ]
    res = run_bass_kernel_spmd(nc, in_maps, core_ids=list(range(8)))
    out = np.empty((B, N, D), dtype=np.float32)
    for b in range(B):
        out[b] = res.results[2 * b]["y"] + res.results[2 * b + 1]["y"] + b_out[None, :]
    return out



# revision 18
# speedup vs baseline: 1.5479x; 1.1958x over previous
"""DSQG attention kernel for 8 Trainium2 NeuronCores.

Sharding: core c = (b, half) with b = c//2 in 0..3, half = c%2.
Each core handles batch b and heads [half*8, half*8+8):
  - qkv+gate projection for its 512 channels (bias folded in via ones-column)
  - 44-tap dyadic attention (24 distinct lags; duplicate-offset pos_bias terms
    folded into per-lag multiplicative weights W[L,h] = sum_dup exp(pos_bias))
  - gated partial output projection through its 512 rows of w_out
Host sums the two half partials per batch and adds b_out.

Shifted k/v windows are fetched per (block, lag) from a zero-padded DRAM
buffer: DRAM rows have no partition-alignment constraint, unlike SBUF reads
(start partition must be 0/32/64/96).
"""
import sys

sys.path.insert(0, "/opt/trn_rl_repo")

import numpy as np
import ml_dtypes

N_SCALES = 11
N_TAPS = 4
OFFSETS = [(1 << j) * tau for j in range(N_SCALES) for tau in range(N_TAPS)]
LAGS = sorted(set(OFFSETS))  # 24 distinct lags
NL = len(LAGS)
B, N, D, H = 4, 4096, 1024, 16
HD = 64
HH = 8  # heads per core
CW = HH * HD  # 512 channels per core
KE = 1152  # padded contraction: 1024 x-cols + 1 ones-col + 127 zero pad
NBLK = N // 128  # 32
PAD = 3072  # zero rows above qkvg for causal lookback
SCALE = HD ** -0.5

# lags needing a DMA window fetch (walrus requires identical start partitions
# on all TensorTensor operands, so only 128-multiples can read SBUF directly)
SMALL = [1, 2, 3, 4, 6, 8, 12, 16, 24, 32, 48, 64, 96, 192]

_CACHE = {}


def _pieces(L):
    """(out_lo, out_hi, tile_delta, src_lo) for lags that are 128-multiples."""
    qd, r = divmod(L, 128)
    assert r == 0, L
    return [(0, 128, qd, 0)]


def _build_program():
    import concourse.bacc as bacc
    import concourse.mybir as mybir
    import concourse.tile as tile
    from concourse.kernels.tile_matmul import matmul_tile_kernel

    bf16 = mybir.dt.bfloat16
    f32 = mybir.dt.float32

    nc = bacc.Bacc("TRN2", target_bir_lowering=False, debug=False, num_devices=8)
    xb = nc.dram_tensor("xb", [N, KE], bf16, kind="ExternalInput").ap()
    wqg = nc.dram_tensor("wqg", [KE, 4 * CW], bf16, kind="ExternalInput").ap()
    wo = nc.dram_tensor("wo", [CW, D], bf16, kind="ExternalInput").ap()
    wrep = nc.dram_tensor("wrep", [128, HH * NL], f32, kind="ExternalInput").ap()
    y = nc.dram_tensor("y", [N, D], f32, kind="ExternalOutput").ap()

    with tile.TileContext(nc) as tc:
        with (
            tc.tile_pool(name="dram", bufs=1, space="DRAM") as dram,
            tc.tile_pool(name="kvpool", bufs=1) as kvpool,
            tc.tile_pool(name="work", bufs=2) as work,
            tc.tile_pool(name="winp", bufs=16) as winp,
            tc.tile_pool(name="prodp", bufs=1) as prodp,
            tc.tile_pool(name="treep", bufs=1) as treep,
            tc.tile_pool(name="constp", bufs=1) as constp,
        ):
            qkvg = dram.tile([PAD + N, 4 * CW], bf16)
            gtil = dram.tile([N, CW], bf16)

            # zero the pad rows (read back by the small-lag window DMAs)
            ztile = constp.tile([128, 2 * CW], bf16, tag="ztile")
            nc.vector.memset(ztile[:], 0.0)
            for t in range(PAD // 128):
                nc.sync.dma_start(out=qkvg[t * 128:(t + 1) * 128, CW:3 * CW], in_=ztile[:])

            # Phase 1: [q|k|v|gate] = x_ext @ w_ext   (biases via ones column)
            matmul_tile_kernel(tc, xb, wqg, qkvg[PAD:, :], transpose_kxm=True)

            wr = constp.tile([128, HH * NL], f32, tag="wr")
            nc.sync.dma_start(out=wr[:], in_=wrep[:])

            # k, v resident in SBUF for the quadrant-alignable lags;
            # sigmoid(gate) precomputed for all blocks (one ACT table load),
            # staged through DRAM to keep SBUF free
            gdram = dram.tile([N, CW], bf16)
            kb = kvpool.tile([128, NBLK * CW], bf16, tag="kb")
            vb = kvpool.tile([128, NBLK * CW], bf16, tag="vb")
            for a in range(NBLK):
                rws = slice(PAD + a * 128, PAD + (a + 1) * 128)
                nc.sync.dma_start(out=kb[:, a * CW:(a + 1) * CW], in_=qkvg[rws, CW:2 * CW])
                nc.sync.dma_start(out=vb[:, a * CW:(a + 1) * CW], in_=qkvg[rws, 2 * CW:3 * CW])
                gt = work.tile([128, CW], bf16, tag="gt")
                nc.sync.dma_start(out=gt[:], in_=qkvg[rws, 3 * CW:4 * CW])
                gs = work.tile([128, CW], bf16, tag="gs")
                nc.scalar.activation(gs[:], gt[:], mybir.ActivationFunctionType.Sigmoid)
                nc.scalar.dma_start(out=gdram[a * 128:(a + 1) * 128, :], in_=gs[:])

            GRP = 1  # blocks per small-lag window fetch group

            def fetch_small_windows(g):
                """One DMA per small lag: k and v windows for GRP blocks.

                Returns {L: tile [128, GRP*2*CW] viewed (p, a_local, kv, c)}."""
                wins = {}
                base = PAD + g * GRP * 128
                for i, L in enumerate(SMALL):
                    win = winp.tile([128, GRP * 2 * CW], bf16, tag="win")
                    eng = nc.sync if (i % 2 == 0) else nc.scalar
                    src = qkvg[base - L:base - L + GRP * 128, CW:3 * CW]
                    eng.dma_start(out=win[:], in_=src)
                    wins[L] = win
                return wins

            def tap_muls(a0, wins, prod, lhs_of, resident, kv, bcast):
                """Emit prod[:, j*CW:(j+1)*CW] = lhs ⊙ window_L for every lag.

                bcast=False (k pass): flat [128, CW] dense muls (2x mode).
                bcast=True (v pass): v channels are (d, h)-ordered, lhs_of(j)
                gives attn [128, HD, HH] with innermost step-1 over h so the
                broadcast mul still hits the packed DVE mode."""
                def r3(ap):
                    return ap.rearrange("p (d h) -> p d h", d=HD)

                al = a0 % GRP
                for j, L in enumerate(LAGS):
                    dst = prod[:, j * CW:(j + 1) * CW]
                    if L in SMALL:
                        w0 = (al * 2 + kv) * CW
                        src = wins[L][:, w0:w0 + CW]
                    else:
                        a = a0 - L // 128
                        if a < 0:
                            nc.vector.memset(dst[:, :], 0.0)
                            continue
                        src = resident[:, a * CW:(a + 1) * CW]
                    if bcast:
                        nc.vector.tensor_mul(r3(dst), lhs_of(j), r3(src))
                    else:
                        nc.vector.tensor_mul(dst, lhs_of(j), src)

            # Phase 2: attention middle, per 128-row block
            wins = None
            for a0 in range(NBLK):
                if a0 % GRP == 0:
                    wins = fetch_small_windows(a0 // GRP)
                n0 = a0 * 128
                rows = slice(PAD + n0, PAD + n0 + 128)
                qb = work.tile([128, CW], bf16, tag="qb")
                nc.sync.dma_start(out=qb[:], in_=qkvg[rows, 0:CW])
                prod = prodp.tile([128, NL * CW], bf16, tag="prod")

                # scores products: prod[:, j*CW:(j+1)*CW] = q * k[n-L]
                tap_muls(a0, wins, prod, lambda j: qb[:], kb, 0, False)
                # reduce over d -> scores [128, (l, h)] via bf16 pairwise tree
                # (tensor_tensor bf16 step-1 runs 2x mode; tensor_reduce is 1x)
                sA = treep.tile([128, 96 * HH * HD // 4], bf16, tag="sA")

                def halve(out_ap, in_ap, G, c):
                    iv = in_ap.rearrange("p (g t c) -> p g t c", g=G, t=2, c=c)
                    nc.vector.tensor_add(
                        out_ap.rearrange("p (g c) -> p g c", g=G, c=c),
                        iv[:, :, 0, :], iv[:, :, 1, :])

                GD = NL * HH  # 192 (l,h) groups
                halve(sA[:, :6144], prod[:], GD, 32)
                halve(prod[:, :3072], sA[:, :6144], GD, 16)
                halve(sA[:, :1536], prod[:, :3072], GD, 8)
                halve(prod[:, :768], sA[:, :1536], GD, 4)
                halve(sA[:, :384], prod[:, :768], GD, 2)
                scores = work.tile([128, NL * HH], f32, tag="scores")
                halve(scores[:], sA[:, :384], GD, 1)
                # softmax over lags with folded pos-bias weights, unnormalized
                ew = work.tile([128, NL * HH], f32, tag="ew")
                nc.scalar.activation(ew[:], scores[:], mybir.ActivationFunctionType.Exp, scale=SCALE)
                ew2 = work.tile([128, NL * HH], f32, tag="ew2")
                nc.vector.tensor_mul(ew2[:], ew[:], wr[:])
                z = work.tile([128, HH], f32, tag="z")
                nc.vector.tensor_reduce(
                    out=z[:].unsqueeze(2),
                    in_=ew2[:].rearrange("p (l h) -> p h l", l=NL, h=HH),
                    axis=mybir.AxisListType.X,
                    op=mybir.AluOpType.add,
                )
                rz = work.tile([128, HH], f32, tag="rz")
                nc.vector.reciprocal(rz[:], z[:])
                attn = work.tile([128, NL * HH], bf16, tag="attn")
                nc.vector.tensor_mul(
                    attn[:].rearrange("p (l h) -> p h l", l=NL, h=HH),
                    ew2[:].rearrange("p (l h) -> p h l", l=NL, h=HH),
                    rz[:].unsqueeze(2).broadcast_to([128, HH, NL]),
                )

                # weighted taps: prod[:, j] = attn[:, j, :] * v[n-L]  (v is (d,h))
                def attn_lhs(j):
                    return attn[:, j * HH:(j + 1) * HH].unsqueeze(1).broadcast_to([128, HD, HH])

                tap_muls(a0, wins, prod, attn_lhs, vb, 1, True)
                # reduce over lags: bf16 pairwise tree over the 24 CW-chunks
                accv = work.tile([128, CW], f32, tag="accv")
                halve(sA[:, :6144], prod[:], 12, CW)
                halve(prod[:, :3072], sA[:, :6144], 6, CW)
                halve(sA[:, :1536], prod[:, :3072], 3, CW)
                nc.vector.tensor_add(prod[:, 0:CW], sA[:, 0:CW], sA[:, CW:2 * CW])
                nc.vector.tensor_add(accv[:], prod[:, 0:CW], sA[:, 2 * CW:3 * CW])
                # gathered*gate (gsig precomputed into gdram)
                gsigt = work.tile([128, CW], bf16, tag="gsigt")
                nc.sync.dma_start(out=gsigt[:], in_=gdram[n0:n0 + 128, :])
                gtl = work.tile([128, CW], bf16, tag="gtl")
                nc.vector.tensor_mul(gtl[:], accv[:], gsigt[:])
                nc.sync.dma_start(out=gtil[n0:n0 + 128, :], in_=gtl[:])

            # Phase 3: y_partial = (gathered*gate) @ w_out[half rows]
            matmul_tile_kernel(tc, gtil[:], wo, y, transpose_kxm=True)

    nc.compile()
    return nc


def _get_program():
    if "nc" not in _CACHE:
        _CACHE["nc"] = _build_program()
    return _CACHE["nc"]


def _core_inputs(x, w_qkv, b_qkv, w_gate, b_gate, w_out, pos_bias, b, half):
    bf = ml_dtypes.bfloat16
    cs = slice(half * CW, (half + 1) * CW)
    # v/gate/w_out channels permuted (h,d) -> (d,h) so the attn-broadcast
    # v-muls have innermost step-1 over h (keeps the DVE 2x packed mode)
    perm = np.array([h * HD + d for d in range(HD) for h in range(HH)])

    xb = np.zeros((N, KE), dtype=bf)
    xb[:, :D] = x[b].astype(bf)
    xb[:, D] = np.asarray(1.0, dtype=bf)

    wqg = np.zeros((KE, 4 * CW), dtype=np.float32)
    wqg[:D, 0:CW] = w_qkv[:, cs]
    wqg[:D, CW:2 * CW] = w_qkv[:, D + cs.start:D + cs.stop]
    wqg[:D, 2 * CW:3 * CW] = w_qkv[:, 2 * D + cs.start:2 * D + cs.stop][:, perm]
    wqg[:D, 3 * CW:4 * CW] = w_gate[:, cs][:, perm]
    wqg[D, 0:CW] = b_qkv[cs]
    wqg[D, CW:2 * CW] = b_qkv[D + cs.start:D + cs.stop]
    wqg[D, 2 * CW:3 * CW] = b_qkv[2 * D + cs.start:2 * D + cs.stop][perm]
    wqg[D, 3 * CW:4 * CW] = b_gate[cs][perm]

    # lag weights: W[j, h] = sum over duplicate offsets of exp(pos_bias[i, h])
    wl = np.zeros((NL, HH), dtype=np.float32)
    for i, off in enumerate(OFFSETS):
        j = LAGS.index(off)
        wl[j, :] += np.exp(pos_bias[i, half * HH:(half + 1) * HH].astype(np.float64)).astype(np.float32)
    wrep = np.broadcast_to(wl.reshape(1, NL * HH), (128, NL * HH)).copy()

    return {
        "xb": xb,
        "wqg": wqg.astype(bf),
        "wo": w_out[cs, :][perm, :].astype(bf),
        "wrep": wrep,
    }


def kernel(x, w_qkv, b_qkv, w_out, b_out, w_gate, b_gate, pos_bias):
    from concourse.bass_utils import run_bass_kernel_spmd

    x = np.asarray(x, dtype=np.float32)
    w_qkv = np.asarray(w_qkv, dtype=np.float32)
    b_qkv = np.asarray(b_qkv, dtype=np.float32)
    w_out = np.asarray(w_out, dtype=np.float32)
    b_out = np.asarray(b_out, dtype=np.float32)
    w_gate = np.asarray(w_gate, dtype=np.float32)
    b_gate = np.asarray(b_gate, dtype=np.float32)
    pos_bias = np.asarray(pos_bias, dtype=np.float32)

    nc = _get_program()
    in_maps = [
        _core_inputs(x, w_qkv, b_qkv, w_gate, b_gate, w_out, pos_bias, c // 2, c % 2)
        for c in range(8)
    ]
    res = run_bass_kernel_spmd(nc, in_maps, core_ids=list(range(8)))
    out = np.empty((B, N, D), dtype=np.float32)
    for b in range(B):
        out[b] = res.results[2 * b]["y"] + res.results[2 * b + 1]["y"] + b_out[None, :]
    return out



# revision 20
# speedup vs baseline: 1.6274x; 1.0514x over previous
"""DSQG attention kernel for 8 Trainium2 NeuronCores.

Sharding: core c = (b, half) with b = c//2 in 0..3, half = c%2.
Each core handles batch b and heads [half*8, half*8+8):
  - qkv+gate projection for its 512 channels (bias folded in via ones-column)
  - 44-tap dyadic attention (24 distinct lags; duplicate-offset pos_bias terms
    folded into per-lag multiplicative weights W[L,h] = sum_dup exp(pos_bias))
  - gated partial output projection through its 512 rows of w_out
Host sums the two half partials per batch and adds b_out.

Shifted k/v windows are fetched per (block, lag) from a zero-padded DRAM
buffer: DRAM rows have no partition-alignment constraint, unlike SBUF reads
(start partition must be 0/32/64/96).
"""
import sys

sys.path.insert(0, "/opt/trn_rl_repo")

import numpy as np
import ml_dtypes

N_SCALES = 11
N_TAPS = 4
OFFSETS = [(1 << j) * tau for j in range(N_SCALES) for tau in range(N_TAPS)]
LAGS = sorted(set(OFFSETS))  # 24 distinct lags
NL = len(LAGS)
B, N, D, H = 4, 4096, 1024, 16
HD = 64
HH = 8  # heads per core
CW = HH * HD  # 512 channels per core
KE = 1152  # padded contraction: 1024 x-cols + 1 ones-col + 127 zero pad
NBLK = N // 128  # 32
PAD = 3072  # zero rows above qkvg for causal lookback
SCALE = HD ** -0.5

# lags needing a DMA window fetch (walrus requires identical start partitions
# on all TensorTensor operands, so only 128-multiples can read SBUF directly)
SMALL = [1, 2, 3, 4, 6, 8, 12, 16, 24, 32, 48, 64, 96, 192]

_CACHE = {}


def _pieces(L):
    """(out_lo, out_hi, tile_delta, src_lo) for lags that are 128-multiples."""
    qd, r = divmod(L, 128)
    assert r == 0, L
    return [(0, 128, qd, 0)]


def _build_program():
    import concourse.bacc as bacc
    import concourse.mybir as mybir
    import concourse.tile as tile
    from concourse.kernels.tile_matmul import matmul_tile_kernel

    bf16 = mybir.dt.bfloat16
    f32 = mybir.dt.float32

    nc = bacc.Bacc("TRN2", target_bir_lowering=False, debug=False, num_devices=8)
    xb = nc.dram_tensor("xb", [N, KE], bf16, kind="ExternalInput").ap()
    wqg = nc.dram_tensor("wqg", [KE, 4 * CW], bf16, kind="ExternalInput").ap()
    wo = nc.dram_tensor("wo", [CW, D], bf16, kind="ExternalInput").ap()
    wrep = nc.dram_tensor("wrep", [128, HH * NL], f32, kind="ExternalInput").ap()
    y = nc.dram_tensor("y", [N, D], f32, kind="ExternalOutput").ap()

    with tile.TileContext(nc) as tc:
        with (
            tc.tile_pool(name="dram", bufs=1, space="DRAM") as dram,
            tc.tile_pool(name="kvpool", bufs=1) as kvpool,
            tc.tile_pool(name="work", bufs=2) as work,
            tc.tile_pool(name="winp", bufs=16) as winp,
            tc.tile_pool(name="prodp", bufs=1) as prodp,
            tc.tile_pool(name="treep", bufs=1) as treep,
            tc.tile_pool(name="constp", bufs=1) as constp,
        ):
            qkvg = dram.tile([PAD + N, 4 * CW], bf16)
            gtil = dram.tile([N, CW], bf16)

            # zero the pad rows (read back by the small-lag window DMAs)
            ztile = constp.tile([128, 2 * CW], bf16, tag="ztile")
            nc.vector.memset(ztile[:], 0.0)
            for t in range(PAD // 128):
                nc.sync.dma_start(out=qkvg[t * 128:(t + 1) * 128, CW:3 * CW], in_=ztile[:])

            # Phase 1: [q|k|v|gate] = x_ext @ w_ext   (biases via ones column)
            matmul_tile_kernel(tc, xb, wqg, qkvg[PAD:, :], transpose_kxm=True)

            wr = constp.tile([128, HH * NL], f32, tag="wr")
            nc.sync.dma_start(out=wr[:], in_=wrep[:])

            # k, v resident in SBUF for the quadrant-alignable lags;
            # sigmoid(gate) precomputed for all blocks (one ACT table load),
            # staged through DRAM to keep SBUF free
            gdram = dram.tile([N, CW], bf16)
            kb = kvpool.tile([128, NBLK * CW], bf16, tag="kb")
            vb = kvpool.tile([128, NBLK * CW], bf16, tag="vb")
            for a in range(NBLK):
                rws = slice(PAD + a * 128, PAD + (a + 1) * 128)
                nc.sync.dma_start(out=kb[:, a * CW:(a + 1) * CW], in_=qkvg[rws, CW:2 * CW])
                nc.sync.dma_start(out=vb[:, a * CW:(a + 1) * CW], in_=qkvg[rws, 2 * CW:3 * CW])
                gt = work.tile([128, CW], bf16, tag="gt")
                nc.sync.dma_start(out=gt[:], in_=qkvg[rws, 3 * CW:4 * CW])
                gs = work.tile([128, CW], bf16, tag="gs")
                nc.scalar.activation(gs[:], gt[:], mybir.ActivationFunctionType.Sigmoid)
                nc.scalar.dma_start(out=gdram[a * 128:(a + 1) * 128, :], in_=gs[:])

            GRP = 1  # blocks per small-lag window fetch group

            def fetch_small_windows(g):
                """One DMA per small lag: k and v windows for GRP blocks.

                Returns {L: tile [128, GRP*2*CW] viewed (p, a_local, kv, c)}."""
                wins = {}
                base = PAD + g * GRP * 128
                for i, L in enumerate(SMALL):
                    win = winp.tile([128, GRP * 2 * CW], bf16, tag="win")
                    eng = nc.sync if (i % 2 == 0) else nc.scalar
                    src = qkvg[base - L:base - L + GRP * 128, CW:3 * CW]
                    eng.dma_start(out=win[:], in_=src)
                    wins[L] = win
                return wins

            def tap_muls(a0, wins, prod, lhs_of, resident, kv, bcast):
                """Emit prod[:, j*CW:(j+1)*CW] = lhs ⊙ window_L for every lag.

                bcast=False (k pass): flat [128, CW] dense muls (2x mode).
                bcast=True (v pass): v channels are (d, h)-ordered, lhs_of(j)
                gives attn [128, HD, HH] with innermost step-1 over h so the
                broadcast mul still hits the packed DVE mode."""
                def r3(ap):
                    return ap.rearrange("p (d h) -> p d h", d=HD)

                al = a0 % GRP
                for j, L in enumerate(LAGS):
                    dst = prod[:, j * CW:(j + 1) * CW]
                    if L in SMALL:
                        w0 = (al * 2 + kv) * CW
                        src = wins[L][:, w0:w0 + CW]
                    else:
                        a = a0 - L // 128
                        if a < 0:
                            continue  # chunk pre-zeroed once; stays zero until live
                        src = resident[:, a * CW:(a + 1) * CW]
                    if bcast:
                        nc.vector.tensor_mul(r3(dst), lhs_of(j), r3(src))
                    else:
                        nc.vector.tensor_mul(dst, lhs_of(j), src)

            # Phase 2: attention middle, per 128-row block
            prod = prodp.tile([128, NL * CW], bf16, tag="prod")
            sA = treep.tile([128, 6144], bf16, tag="sA")
            # pre-zero chunks of not-yet-live lags once; the missing set only
            # shrinks with a0 and tap_muls writes each chunk when it goes live
            dead = [j for j, L in enumerate(LAGS) if L not in SMALL and L >= 128]
            runs = []
            for j in dead:
                if runs and runs[-1][1] == j:
                    runs[-1][1] = j + 1
                else:
                    runs.append([j, j + 1])
            for j0, j1 in runs:
                nc.vector.memset(prod[:, j0 * CW:j1 * CW], 0.0)

            def halve(out_ap, in_ap, G, c):
                iv = in_ap.rearrange("p (g t c) -> p g t c", g=G, t=2, c=c)
                nc.vector.tensor_add(
                    out_ap.rearrange("p (g c) -> p g c", g=G, c=c),
                    iv[:, :, 0, :], iv[:, :, 1, :])

            GD = NL * HH  # 192 (l,h) groups
            wins = None
            for a0 in range(NBLK):
                if a0 % GRP == 0:
                    wins = fetch_small_windows(a0 // GRP)
                n0 = a0 * 128
                rows = slice(PAD + n0, PAD + n0 + 128)
                qb = work.tile([128, CW], bf16, tag="qb")
                nc.sync.dma_start(out=qb[:], in_=qkvg[rows, 0:CW])

                # scores products: prod[:, j*CW:(j+1)*CW] = q * k[n-L]
                tap_muls(a0, wins, prod, lambda j: qb[:], kb, 0, False)
                # reduce over d -> scores [128, (l, h)] via bf16 pairwise tree
                # (tensor_tensor bf16 step-1 runs 2x mode; tensor_reduce is 1x)
                halve(sA[:, :6144], prod[:], GD, 32)
                halve(prod[:, :3072], sA[:, :6144], GD, 16)
                halve(sA[:, :1536], prod[:, :3072], GD, 8)
                halve(prod[:, :768], sA[:, :1536], GD, 4)
                halve(sA[:, :384], prod[:, :768], GD, 2)
                scores = work.tile([128, NL * HH], f32, tag="scores")
                halve(scores[:], sA[:, :384], GD, 1)
                # softmax over lags with folded pos-bias weights, unnormalized
                ew = work.tile([128, NL * HH], f32, tag="ew")
                nc.scalar.activation(ew[:], scores[:], mybir.ActivationFunctionType.Exp, scale=SCALE)
                ew2 = work.tile([128, NL * HH], f32, tag="ew2")
                nc.vector.tensor_mul(ew2[:], ew[:], wr[:])
                z = work.tile([128, HH], f32, tag="z")
                nc.vector.tensor_reduce(
                    out=z[:].unsqueeze(2),
                    in_=ew2[:].rearrange("p (l h) -> p h l", l=NL, h=HH),
                    axis=mybir.AxisListType.X,
                    op=mybir.AluOpType.add,
                )
                rz = work.tile([128, HH], f32, tag="rz")
                nc.vector.reciprocal(rz[:], z[:])
                attn = work.tile([128, NL * HH], bf16, tag="attn")
                nc.vector.tensor_mul(
                    attn[:].rearrange("p (l h) -> p h l", l=NL, h=HH),
                    ew2[:].rearrange("p (l h) -> p h l", l=NL, h=HH),
                    rz[:].unsqueeze(2).broadcast_to([128, HH, NL]),
                )

                # weighted taps: prod[:, j] = attn[:, j, :] * v[n-L]  (v is (d,h))
                def attn_lhs(j):
                    return attn[:, j * HH:(j + 1) * HH].unsqueeze(1).broadcast_to([128, HD, HH])

                tap_muls(a0, wins, prod, attn_lhs, vb, 1, True)
                # reduce over lags: bf16 pairwise tree over the 24 CW-chunks
                accv = work.tile([128, CW], f32, tag="accv")
                halve(sA[:, :6144], prod[:], 12, CW)
                halve(prod[:, :3072], sA[:, :6144], 6, CW)
                halve(sA[:, :1536], prod[:, :3072], 3, CW)
                nc.vector.tensor_add(prod[:, 0:CW], sA[:, 0:CW], sA[:, CW:2 * CW])
                nc.vector.tensor_add(accv[:], prod[:, 0:CW], sA[:, 2 * CW:3 * CW])
                # gathered*gate (gsig precomputed into gdram)
                gsigt = work.tile([128, CW], bf16, tag="gsigt")
                nc.sync.dma_start(out=gsigt[:], in_=gdram[n0:n0 + 128, :])
                gtl = work.tile([128, CW], bf16, tag="gtl")
                nc.vector.tensor_mul(gtl[:], accv[:], gsigt[:])
                nc.sync.dma_start(out=gtil[n0:n0 + 128, :], in_=gtl[:])

            # Phase 3: y_partial = (gathered*gate) @ w_out[half rows]
            matmul_tile_kernel(tc, gtil[:], wo, y, transpose_kxm=True)

    nc.compile()
    return nc


def _get_program():
    if "nc" not in _CACHE:
        _CACHE["nc"] = _build_program()
    return _CACHE["nc"]


def _core_inputs(x, w_qkv, b_qkv, w_gate, b_gate, w_out, pos_bias, b, half):
    bf = ml_dtypes.bfloat16
    cs = slice(half * CW, (half + 1) * CW)
    # v/gate/w_out channels permuted (h,d) -> (d,h) so the attn-broadcast
    # v-muls have innermost step-1 over h (keeps the DVE 2x packed mode)
    perm = np.array([h * HD + d for d in range(HD) for h in range(HH)])

    xb = np.zeros((N, KE), dtype=bf)
    xb[:, :D] = x[b].astype(bf)
    xb[:, D] = np.asarray(1.0, dtype=bf)

    wqg = np.zeros((KE, 4 * CW), dtype=np.float32)
    wqg[:D, 0:CW] = w_qkv[:, cs]
    wqg[:D, CW:2 * CW] = w_qkv[:, D + cs.start:D + cs.stop]
    wqg[:D, 2 * CW:3 * CW] = w_qkv[:, 2 * D + cs.start:2 * D + cs.stop][:, perm]
    wqg[:D, 3 * CW:4 * CW] = w_gate[:, cs][:, perm]
    wqg[D, 0:CW] = b_qkv[cs]
    wqg[D, CW:2 * CW] = b_qkv[D + cs.start:D + cs.stop]
    wqg[D, 2 * CW:3 * CW] = b_qkv[2 * D + cs.start:2 * D + cs.stop][perm]
    wqg[D, 3 * CW:4 * CW] = b_gate[cs][perm]

    # lag weights: W[j, h] = sum over duplicate offsets of exp(pos_bias[i, h])
    wl = np.zeros((NL, HH), dtype=np.float32)
    for i, off in enumerate(OFFSETS):
        j = LAGS.index(off)
        wl[j, :] += np.exp(pos_bias[i, half * HH:(half + 1) * HH].astype(np.float64)).astype(np.float32)
    wrep = np.broadcast_to(wl.reshape(1, NL * HH), (128, NL * HH)).copy()

    return {
        "xb": xb,
        "wqg": wqg.astype(bf),
        "wo": w_out[cs, :][perm, :].astype(bf),
        "wrep": wrep,
    }


def kernel(x, w_qkv, b_qkv, w_out, b_out, w_gate, b_gate, pos_bias):
    from concourse.bass_utils import run_bass_kernel_spmd

    x = np.asarray(x, dtype=np.float32)
    w_qkv = np.asarray(w_qkv, dtype=np.float32)
    b_qkv = np.asarray(b_qkv, dtype=np.float32)
    w_out = np.asarray(w_out, dtype=np.float32)
    b_out = np.asarray(b_out, dtype=np.float32)
    w_gate = np.asarray(w_gate, dtype=np.float32)
    b_gate = np.asarray(b_gate, dtype=np.float32)
    pos_bias = np.asarray(pos_bias, dtype=np.float32)

    nc = _get_program()
    in_maps = [
        _core_inputs(x, w_qkv, b_qkv, w_gate, b_gate, w_out, pos_bias, c // 2, c % 2)
        for c in range(8)
    ]
    res = run_bass_kernel_spmd(nc, in_maps, core_ids=list(range(8)))
    out = np.empty((B, N, D), dtype=np.float32)
    for b in range(B):
        out[b] = res.results[2 * b]["y"] + res.results[2 * b + 1]["y"] + b_out[None, :]
    return out



# revision 26
# speedup vs baseline: 1.8433x; 1.1327x over previous
"""DSQG attention kernel for 8 Trainium2 NeuronCores.

Sharding: core c = (b, half) with b = c//2 in 0..3, half = c%2.
Each core handles batch b and heads [half*8, half*8+8):
  - qkv+gate projection for its 512 channels (bias folded in via ones-column)
  - 44-tap dyadic attention (24 distinct lags; duplicate-offset pos_bias terms
    folded into per-lag multiplicative weights W[L,h] = sum_dup exp(pos_bias))
  - gated partial output projection through its 512 rows of w_out
Host sums the two half partials per batch and adds b_out.

Shifted k/v windows are fetched per (block, lag) from a zero-padded DRAM
buffer: DRAM rows have no partition-alignment constraint, unlike SBUF reads
(start partition must be 0/32/64/96).
"""
import sys

sys.path.insert(0, "/opt/trn_rl_repo")

import numpy as np
import ml_dtypes

N_SCALES = 11
N_TAPS = 4
OFFSETS = [(1 << j) * tau for j in range(N_SCALES) for tau in range(N_TAPS)]
LAGS = sorted(set(OFFSETS))  # 24 distinct lags
NL = len(LAGS)
B, N, D, H = 4, 4096, 1024, 16
HD = 64
HH = 8  # heads per core
CW = HH * HD  # 512 channels per core
KE = 1152  # padded contraction: 1024 x-cols + 1 ones-col + 127 zero pad
NBLK = N // 128  # 32
PAD = 3072  # zero rows above qkvg for causal lookback
SCALE = HD ** -0.5

# lags needing a DMA window fetch (walrus requires identical start partitions
# on all TensorTensor operands, so only 128-multiples can read SBUF directly)
SMALL = [1, 2, 3, 4, 6, 8, 12, 16, 24, 32, 48, 64, 96, 192]

_CACHE = {}


def _pieces(L):
    """(out_lo, out_hi, tile_delta, src_lo) for lags that are 128-multiples."""
    qd, r = divmod(L, 128)
    assert r == 0, L
    return [(0, 128, qd, 0)]


def _build_program():
    import concourse.bacc as bacc
    import concourse.mybir as mybir
    import concourse.tile as tile
    from concourse.kernels.tile_matmul import matmul_tile_kernel

    bf16 = mybir.dt.bfloat16
    f32 = mybir.dt.float32

    nc = bacc.Bacc("TRN2", target_bir_lowering=False, debug=False, num_devices=8)
    xb = nc.dram_tensor("xb", [KE, N], bf16, kind="ExternalInput").ap()
    wqg = nc.dram_tensor("wqg", [KE, 4 * CW], bf16, kind="ExternalInput").ap()
    wo = nc.dram_tensor("wo", [CW, D], bf16, kind="ExternalInput").ap()
    wrep = nc.dram_tensor("wrep", [128, HH * NL], bf16, kind="ExternalInput").ap()
    y = nc.dram_tensor("y", [N, D], f32, kind="ExternalOutput").ap()

    with tile.TileContext(nc) as tc:
        with (
            tc.tile_pool(name="dram", bufs=1, space="DRAM") as dram,
            tc.tile_pool(name="kvpool", bufs=1) as kvpool,
            tc.tile_pool(name="work", bufs=2) as work,
            tc.tile_pool(name="winp", bufs=16) as winp,
            tc.tile_pool(name="prodp", bufs=1) as prodp,
            tc.tile_pool(name="treep", bufs=1) as treep,
            tc.tile_pool(name="constp", bufs=1) as constp,
        ):
            qkvg = dram.tile([PAD + N, 4 * CW], bf16)
            gtil = dram.tile([N, CW], bf16)

            # zero the pad rows (read back by the small-lag window DMAs)
            ztile = constp.tile([128, 2 * CW], bf16, tag="ztile")
            nc.vector.memset(ztile[:], 0.0)
            for t in range(PAD // 128):
                nc.sync.dma_start(out=qkvg[t * 128:(t + 1) * 128, CW:3 * CW], in_=ztile[:])

            # Phase 1: [q|k|v|gate] = x_ext @ w_ext   (biases via ones column;
            # xb arrives host-transposed so no kxm DMA transposes)
            matmul_tile_kernel(tc, xb, wqg, qkvg[PAD:, :])

            wr = constp.tile([128, HH * NL], bf16, tag="wr")
            nc.sync.dma_start(out=wr[:], in_=wrep[:])

            # k, v resident in SBUF for the quadrant-alignable lags;
            # sigmoid(gate) precomputed for all blocks (one ACT table load),
            # staged through DRAM to keep SBUF free
            gdram = dram.tile([N, CW], bf16)
            kb = kvpool.tile([128, NBLK * CW], bf16, tag="kb")
            vb = kvpool.tile([128, NBLK * CW], bf16, tag="vb")
            for a in range(NBLK):
                rws = slice(PAD + a * 128, PAD + (a + 1) * 128)
                nc.sync.dma_start(out=kb[:, a * CW:(a + 1) * CW], in_=qkvg[rws, CW:2 * CW])
                nc.sync.dma_start(out=vb[:, a * CW:(a + 1) * CW], in_=qkvg[rws, 2 * CW:3 * CW])
                gt = work.tile([128, CW], bf16, tag="gt")
                nc.sync.dma_start(out=gt[:], in_=qkvg[rws, 3 * CW:4 * CW])
                gs = work.tile([128, CW], bf16, tag="gs")
                nc.scalar.activation(gs[:], gt[:], mybir.ActivationFunctionType.Sigmoid)
                nc.scalar.dma_start(out=gdram[a * 128:(a + 1) * 128, :], in_=gs[:])

            GRP = 1  # blocks per small-lag window fetch group

            def fetch_small_windows(g):
                """One DMA per small lag: k and v windows for GRP blocks.

                Returns {L: tile [128, GRP*2*CW] viewed (p, a_local, kv, c)}."""
                wins = {}
                base = PAD + g * GRP * 128
                for i, L in enumerate(SMALL):
                    win = winp.tile([128, GRP * 2 * CW], bf16, tag="win")
                    eng = nc.sync if (i % 2 == 0) else nc.scalar
                    src = qkvg[base - L:base - L + GRP * 128, CW:3 * CW]
                    eng.dma_start(out=win[:], in_=src)
                    wins[L] = win
                return wins

            def tap_muls(a0, wins, prod, lhs_of, resident, kv, bcast):
                """Emit prod[:, j*CW:(j+1)*CW] = lhs ⊙ window_L for every lag.

                bcast=False (k pass): flat [128, CW] dense muls (2x mode).
                bcast=True (v pass): v channels are (d, h)-ordered, lhs_of(j)
                gives attn [128, HD, HH] with innermost step-1 over h so the
                broadcast mul still hits the packed DVE mode."""
                def r3(ap):
                    return ap.rearrange("p (d h) -> p d h", d=HD)

                al = a0 % GRP
                for j, L in enumerate(LAGS):
                    dst = prod[:, j * CW:(j + 1) * CW]
                    if L in SMALL:
                        w0 = (al * 2 + kv) * CW
                        src = wins[L][:, w0:w0 + CW]
                    else:
                        a = a0 - L // 128
                        if a < 0:
                            continue  # chunk pre-zeroed once; stays zero until live
                        src = resident[:, a * CW:(a + 1) * CW]
                    if bcast:
                        nc.vector.tensor_mul(r3(dst), lhs_of(j), r3(src))
                    else:
                        nc.vector.tensor_mul(dst, lhs_of(j), src)

            # Phase 2: attention middle, per 128-row block
            prod = prodp.tile([128, NL * CW], bf16, tag="prod")
            sA = treep.tile([128, 6144], bf16, tag="sA")
            # pre-zero chunks of not-yet-live lags once; the missing set only
            # shrinks with a0 and tap_muls writes each chunk when it goes live
            dead = [j for j, L in enumerate(LAGS) if L not in SMALL and L >= 128]
            runs = []
            for j in dead:
                if runs and runs[-1][1] == j:
                    runs[-1][1] = j + 1
                else:
                    runs.append([j, j + 1])
            for j0, j1 in runs:
                nc.vector.memset(prod[:, j0 * CW:j1 * CW], 0.0)

            def halve(out_ap, in_ap, G, c):
                iv = in_ap.rearrange("p (g t c) -> p g t c", g=G, t=2, c=c)
                nc.vector.tensor_add(
                    out_ap.rearrange("p (g c) -> p g c", g=G, c=c),
                    iv[:, :, 0, :], iv[:, :, 1, :])

            GD = NL * HH  # 192 (l,h) groups
            wins = None
            for a0 in range(NBLK):
                if a0 % GRP == 0:
                    wins = fetch_small_windows(a0 // GRP)
                n0 = a0 * 128
                rows = slice(PAD + n0, PAD + n0 + 128)
                qb = work.tile([128, CW], bf16, tag="qb")
                nc.sync.dma_start(out=qb[:], in_=qkvg[rows, 0:CW])

                # scores products: prod[:, j*CW:(j+1)*CW] = q * k[n-L]
                tap_muls(a0, wins, prod, lambda j: qb[:], kb, 0, False)
                # reduce over d -> scores [128, (l, h)] via bf16 pairwise tree
                # (tensor_tensor bf16 step-1 runs 2x mode; tensor_reduce is 1x)
                halve(sA[:, :6144], prod[:], GD, 32)
                halve(prod[:, :3072], sA[:, :6144], GD, 16)
                halve(sA[:, :1536], prod[:, :3072], GD, 8)
                halve(prod[:, :768], sA[:, :1536], GD, 4)
                halve(sA[:, :384], prod[:, :768], GD, 2)
                scores = work.tile([128, NL * HH], f32, tag="scores")
                halve(scores[:], sA[:, :384], GD, 1)
                # unnormalized weights u = exp(s*scale) * W; 1/z is folded into
                # the gate stage so softmax normalization leaves the hot path
                ew = work.tile([128, NL * HH], bf16, tag="ew")
                nc.scalar.activation(ew[:], scores[:], mybir.ActivationFunctionType.Exp, scale=SCALE)
                ub = work.tile([128, NL * HH], bf16, tag="ub")
                nc.vector.tensor_mul(ub[:], ew[:], wr[:])
                z = work.tile([128, HH], f32, tag="z")
                nc.vector.tensor_reduce(
                    out=z[:].unsqueeze(2),
                    in_=ub[:].rearrange("p (l h) -> p h l", l=NL, h=HH),
                    axis=mybir.AxisListType.X,
                    op=mybir.AluOpType.add,
                )
                rz = work.tile([128, HH], f32, tag="rz")
                nc.vector.reciprocal(rz[:], z[:])
                rzb = work.tile([128, HH], bf16, tag="rzb")
                nc.vector.tensor_copy(rzb[:], rz[:])

                # weighted taps: prod[:, j] = u[:, j, :] * v[n-L]  (v is (d,h))
                def attn_lhs(j):
                    return ub[:, j * HH:(j + 1) * HH].unsqueeze(1).broadcast_to([128, HD, HH])

                tap_muls(a0, wins, prod, attn_lhs, vb, 1, True)
                # reduce over lags: bf16 pairwise tree over the 24 CW-chunks
                accv = work.tile([128, CW], bf16, tag="accv")
                halve(sA[:, :6144], prod[:], 12, CW)
                halve(prod[:, :3072], sA[:, :6144], 6, CW)
                halve(sA[:, :1536], prod[:, :3072], 3, CW)
                nc.vector.tensor_add(prod[:, 0:CW], sA[:, 0:CW], sA[:, CW:2 * CW])
                nc.vector.tensor_add(accv[:], prod[:, 0:CW], sA[:, 2 * CW:3 * CW])
                # normalize by 1/z and gate (gsig precomputed into gdram)
                gsigt = work.tile([128, CW], bf16, tag="gsigt")
                nc.sync.dma_start(out=gsigt[:], in_=gdram[n0:n0 + 128, :])
                gtl1 = work.tile([128, CW], bf16, tag="gtl1")
                nc.vector.tensor_mul(
                    gtl1[:].rearrange("p (d h) -> p d h", d=HD),
                    accv[:].rearrange("p (d h) -> p d h", d=HD),
                    rzb[:].unsqueeze(1).broadcast_to([128, HD, HH]),
                )
                gtl = work.tile([128, CW], bf16, tag="gtl")
                nc.vector.tensor_mul(gtl[:], gtl1[:], gsigt[:])
                nc.sync.dma_start(out=gtil[n0:n0 + 128, :], in_=gtl[:])

            # Phase 3: y_partial = (gathered*gate) @ w_out[half rows]
            matmul_tile_kernel(tc, gtil[:], wo, y, transpose_kxm=True)

    nc.compile()
    return nc


def _get_program():
    if "nc" not in _CACHE:
        _CACHE["nc"] = _build_program()
    return _CACHE["nc"]


def _core_inputs(x, w_qkv, b_qkv, w_gate, b_gate, w_out, pos_bias, b, half):
    bf = ml_dtypes.bfloat16
    cs = slice(half * CW, (half + 1) * CW)
    # v/gate/w_out channels permuted (h,d) -> (d,h) so the attn-broadcast
    # v-muls have innermost step-1 over h (keeps the DVE 2x packed mode)
    perm = np.array([h * HD + d for d in range(HD) for h in range(HH)])

    xb = np.zeros((KE, N), dtype=bf)
    xb[:D, :] = x[b].T.astype(bf)
    xb[D, :] = np.asarray(1.0, dtype=bf)

    wqg = np.zeros((KE, 4 * CW), dtype=np.float32)
    wqg[:D, 0:CW] = w_qkv[:, cs]
    wqg[:D, CW:2 * CW] = w_qkv[:, D + cs.start:D + cs.stop]
    wqg[:D, 2 * CW:3 * CW] = w_qkv[:, 2 * D + cs.start:2 * D + cs.stop][:, perm]
    wqg[:D, 3 * CW:4 * CW] = w_gate[:, cs][:, perm]
    wqg[D, 0:CW] = b_qkv[cs]
    wqg[D, CW:2 * CW] = b_qkv[D + cs.start:D + cs.stop]
    wqg[D, 2 * CW:3 * CW] = b_qkv[2 * D + cs.start:2 * D + cs.stop][perm]
    wqg[D, 3 * CW:4 * CW] = b_gate[cs][perm]

    # lag weights: W[j, h] = sum over duplicate offsets of exp(pos_bias[i, h])
    wl = np.zeros((NL, HH), dtype=np.float32)
    for i, off in enumerate(OFFSETS):
        j = LAGS.index(off)
        wl[j, :] += np.exp(pos_bias[i, half * HH:(half + 1) * HH].astype(np.float64)).astype(np.float32)
    wrep = np.broadcast_to(wl.reshape(1, NL * HH), (128, NL * HH)).astype(bf).copy()

    return {
        "xb": xb,
        "wqg": wqg.astype(bf),
        "wo": w_out[cs, :][perm, :].astype(bf),
        "wrep": wrep,
    }


def kernel(x, w_qkv, b_qkv, w_out, b_out, w_gate, b_gate, pos_bias):
    from concourse.bass_utils import run_bass_kernel_spmd

    x = np.asarray(x, dtype=np.float32)
    w_qkv = np.asarray(w_qkv, dtype=np.float32)
    b_qkv = np.asarray(b_qkv, dtype=np.float32)
    w_out = np.asarray(w_out, dtype=np.float32)
    b_out = np.asarray(b_out, dtype=np.float32)
    w_gate = np.asarray(w_gate, dtype=np.float32)
    b_gate = np.asarray(b_gate, dtype=np.float32)
    pos_bias = np.asarray(pos_bias, dtype=np.float32)

    nc = _get_program()
    in_maps = [
        _core_inputs(x, w_qkv, b_qkv, w_gate, b_gate, w_out, pos_bias, c // 2, c % 2)
        for c in range(8)
    ]
    res = run_bass_kernel_spmd(nc, in_maps, core_ids=list(range(8)))
    out = np.empty((B, N, D), dtype=np.float32)
    for b in range(B):
        out[b] = res.results[2 * b]["y"] + res.results[2 * b + 1]["y"] + b_out[None, :]
    return out

